# revision 40
# baseline (speedup 1.0000x reference)
"""Trainium2 Bass kernel for nn_Attention_48687749268214.

Self-attention with pair-bias. v7 architecture:

- Sequence-shard i across 8 cores (64 query rows each); j rolled per core so
  own q rows sit at local rows 0:63 (no collectives).
- pair is HOST-TRANSPOSED to [c=768, ij=32768] and PRE-SCALED by 2^-16
  (exact in bf16): the bias einsum runs as wg'(=wg*2^16)^T @ pairT on the PE
  with no on-device transposes and no per-element LN normalize.
- LN fold: bias[ij,h] = r[ij]*(raw[ij,h] - mu'[ij]*wsum'[h]); +sb[h] is
  constant over j and cancels in softmax -> dropped. Scale factors cancel.
- colsum rides the bias matmul group (weight col 12 = ones); sumsq rides as
  col 13 (weight 2^-64) over bit-trick-squared pair.
- Square is ONE bitwise DVE op: sq_bits = (bits & 0x7FFF) << 1, which equals
  x^2 * 2^(127-2*16) up to mantissa error; the systematic -3.8% mantissa bias
  and all scale factors fold into host constants calrc/epsP/expb.
- Bias panels [14,512]: ACT copies PSUM->SBUF one group late (software
  pipelined, no ACT stalls), then ACT-issued SBUF DMA collapses into
  braw [64, 14, 512].
- Pair DMA split across sync/scalar/gpsimd rings; x-path and first-half
  attention emission interleaved between pair groups to keep every in-order
  engine program free of long foreign blobs.
"""

import sys

sys.path.insert(0, "/opt/trn_rl_repo")

import math
from contextlib import ExitStack

import ml_dtypes
import numpy as np

import concourse.bass as bass
import concourse.tile as tile
from concourse import bacc, mybir
from concourse.bass_utils import run_bass_kernel_spmd
from concourse.masks import make_identity

F32 = mybir.dt.float32
BF16 = mybir.dt.bfloat16
U16 = mybir.dt.uint16
AF = mybir.ActivationFunctionType
OP = mybir.AluOpType

C = 768
H = 12
HD = 64
N = 512
NCORES = 8
IB = N // NCORES  # 64 i rows per core
NIJ = IB * N
EPS = 1e-5
RC = 1.0 / C
G = 1024  # ij columns per stream group (2 i-rows)
NG = NIJ // G
RPG = G // N  # i-rows per group
PSC = 16  # pair pre-scale: pair * 2^-PSC

bf = ml_dtypes.bfloat16


def _build():
    nc = bacc.Bacc(
        "TRN2", target_bir_lowering=False, debug=False, num_devices=NCORES
    )

    pairT_d = nc.dram_tensor("pairT_s", [NG * 128, 6 * G], BF16, kind="ExternalInput").ap()
    x_d = nc.dram_tensor("x_s", [N, C], BF16, kind="ExternalInput").ap()
    wqkvt_d = nc.dram_tensor("wqkvt", [C, 3 * C], BF16, kind="ExternalInput").ap()
    bqkv_d = nc.dram_tensor("bqkv", [1, 3 * C], BF16, kind="ExternalInput").ap()
    wprojt_d = nc.dram_tensor("wprojt", [C, C], BF16, kind="ExternalInput").ap()
    bproj_d = nc.dram_tensor("bproj", [1, C], BF16, kind="ExternalInput").ap()
    wgx_d = nc.dram_tensor("wgx", [C, 14], BF16, kind="ExternalInput").ap()
    wsum_d = nc.dram_tensor("wsum16", [1, 16], F32, kind="ExternalInput").ap()
    cal_d = nc.dram_tensor("calrc", [1, 1], F32, kind="ExternalInput").ap()
    reps_d = nc.dram_tensor("lnreps", [6, C], BF16, kind="ExternalInput").ap()
    out_d = nc.dram_tensor("out", [IB, C], F32, kind="ExternalOutput").ap()

    with tile.TileContext(nc) as tc, ExitStack() as ctx:
        sing = ctx.enter_context(tc.tile_pool(name="sing", bufs=1))
        pairp = ctx.enter_context(tc.tile_pool(name="pairp", bufs=3))
        sqp = ctx.enter_context(tc.tile_pool(name="sqp", bufs=2))
        stagep = ctx.enter_context(tc.tile_pool(name="stagep", bufs=3))
        statp = ctx.enter_context(tc.tile_pool(name="statp", bufs=1))
        dumpp = ctx.enter_context(tc.tile_pool(name="dumpp", bufs=2))
        fixp = ctx.enter_context(tc.tile_pool(name="fixp", bufs=1))
        attnp = ctx.enter_context(tc.tile_pool(name="attnp", bufs=2))
        ps_t = ctx.enter_context(tc.tile_pool(name="ps_t", bufs=2, space="PSUM"))
        ps_mm = ctx.enter_context(tc.tile_pool(name="ps_mm", bufs=2, space="PSUM"))
        ps_b = ctx.enter_context(tc.tile_pool(name="ps_b", bufs=4, space="PSUM"))

        # ---- singles / weights ----
        id128 = sing.tile([128, 128], BF16)
        make_identity(nc, id128)
        ones_col = sing.tile([1, 128], BF16)
        nc.vector.memset(ones_col, 1.0)
        epsT = sing.tile([128, 1], F32)
        nc.vector.memset(epsT, EPS)
        epsP = sing.tile([IB, 1], F32)
        nc.vector.memset(epsP, EPS * 2.0 ** (-2 * PSC))
        expb = sing.tile([IB, 1], F32)
        nc.vector.memset(expb, -PSC * math.log(2.0))

        wqkvt = sing.tile([128, 6, 3 * C], BF16)
        wprojt = sing.tile([128, 6, C], BF16)

        def load_w1():
            nc.gpsimd.dma_start(
                out=wqkvt, in_=wqkvt_d.rearrange("(k p) o -> p k o", p=128)
            )

        def load_w2():
            nc.gpsimd.dma_start(
                out=wprojt, in_=wprojt_d.rearrange("(k p) o -> p k o", p=128)
            )
        wgx = sing.tile([128, 6, 14], BF16)
        nc.sync.dma_start(out=wgx, in_=wgx_d.rearrange("(k p) o -> p k o", p=128))
        sqsel = sing.tile([128, 14], BF16)
        nc.vector.memset(sqsel, 0.0)
        nc.vector.memset(sqsel[:, 13:14], 2.0 ** -64)
        bqkv = sing.tile([1, 3 * C], BF16)
        nc.gpsimd.dma_start(out=bqkv, in_=bqkv_d)
        bproj = sing.tile([1, C], BF16)
        nc.gpsimd.dma_start(out=bproj, in_=bproj_d)
        x_sb = sing.tile([128, 4, C], BF16)

        def load_x():
            nc.scalar.dma_start(
                out=x_sb, in_=x_d.rearrange("(t p) c -> p t c", p=128)
            )
        wsrep = sing.tile([IB, 16], F32)
        nc.gpsimd.dma_start(
            out=wsrep,
            in_=bass.AP(tensor=wsum_d.tensor, offset=0, ap=[[0, IB], [1, 16]]),
        )
        calT = sing.tile([IB, 1], F32)
        nc.gpsimd.dma_start(
            out=calT,
            in_=bass.AP(tensor=cal_d.tensor, offset=0, ap=[[0, IB], [1, 1]]),
        )
        reps = sing.tile([128, 6, C], BF16)
        for rI in range(6):
            nc.gpsimd.dma_start(
                out=reps[:, rI, :],
                in_=bass.AP(
                    tensor=reps_d.tensor, offset=rI * C, ap=[[0, 128], [1, C]]
                ),
            )

        def ln_stats(src_ap, s1_ap, s2_ap, dA, dB):
            nc.vector.tensor_scalar(
                out=dA, in0=src_ap, scalar1=1.0, scalar2=0.0, op0=OP.mult,
                op1=OP.add, accum_out=s1_ap,
            )
            nc.scalar.activation(out=dB, in_=src_ap, func=AF.Square, accum_out=s2_ap)

        def ln_mu_r(s1, s2, mu, r, tmp, w):
            nc.vector.tensor_scalar(
                out=mu[:, 0:w], in0=s1[:, 0:w], scalar1=RC, scalar2=None, op0=OP.mult
            )
            nc.vector.tensor_tensor(
                out=tmp[:, 0:w], in0=mu[:, 0:w], in1=mu[:, 0:w], op=OP.mult
            )
            nc.vector.scalar_tensor_tensor(
                out=tmp[:, 0:w], in0=s2[:, 0:w], scalar=RC, in1=tmp[:, 0:w],
                op0=OP.mult, op1=OP.subtract,
            )
            nc.scalar.activation(
                out=tmp[:, 0:w], in_=tmp[:, 0:w], func=AF.Ln, bias=epsT
            )
            nc.scalar.activation(out=r[:, 0:w], in_=tmp[:, 0:w], func=AF.Exp, scale=-0.5)

        # ================= pair group pipeline =================
        braw = sing.tile([IB, 14, N], BF16)
        pend = []

        def flush_pend():
            while pend:
                i, psA = pend.pop(0)
                stg = stagep.tile([14, N], BF16, tag="stg")
                nc.scalar.copy(out=stg, in_=psA[0:14, :])
                nc.scalar.dma_start(out=braw[i : i + 1, :, :], in_=stg)

        def do_group(g):
            P = pairp.tile([128, 6, G], BF16, tag="P")
            pg = pairT_d[g * 128 : (g + 1) * 128, :]
            nc.sync.dma_start(out=P[:, 0:3, :], in_=pg[:, 0 : 3 * G])
            nc.scalar.dma_start(out=P[:, 3:5, :], in_=pg[:, 3 * G : 5 * G])
            nc.gpsimd.dma_start(out=P[:, 5:6, :], in_=pg[:, 5 * G : 6 * G])
            SQ = sqp.tile([128, 6, G], BF16, tag="SQ")
            Pb = P.bitcast(U16).rearrange("p a b -> p (a b)")
            SQb = SQ.bitcast(U16).rearrange("p a b -> p (a b)")
            nc.vector.tensor_scalar(
                out=SQb, in0=Pb, scalar1=32767, scalar2=1,
                op0=OP.bitwise_and, op1=OP.logical_shift_left,
            )
            flush_pend()
            for q in range(RPG):
                i = g * RPG + q
                psA = ps_b.tile([128, N], F32, tag="bias")
                for ch in range(6):
                    nc.tensor.matmul(
                        psA[0:14, :],
                        lhsT=wgx[:, ch, :],
                        rhs=P[:, ch, q * N : (q + 1) * N],
                        start=(ch == 0), stop=False,
                    )
                for ch in range(6):
                    nc.tensor.matmul(
                        psA[0:14, :],
                        lhsT=sqsel,
                        rhs=SQ[:, ch, q * N : (q + 1) * N],
                        start=False, stop=(ch == 5),
                    )
                pend.append((i, psA))

        # ================= x path stages =================
        xn = sing.tile([128, 4, C], BF16)
        xnT = sing.tile([128, 6, N], BF16)
        qkv = sing.tile([128, 4, 3 * C], BF16)
        kT = sing.tile([128, 6, N], BF16)
        qT = sing.tile([128, 6, IB], BF16)
        s1x = statp.tile([128, 4], F32, tag="s1")
        s2x = statp.tile([128, 4], F32, tag="s2")
        mux = statp.tile([128, 4], F32, tag="mu")
        rx = statp.tile([128, 4], F32, tag="r")
        tmpx = statp.tile([128, 4], F32, tag="tmp")
        s1q = statp.tile([128, 8], F32, tag="s1q")
        s2q = statp.tile([128, 8], F32, tag="s2q")
        muq = statp.tile([128, 8], F32, tag="muq")
        rq = statp.tile([128, 8], F32, tag="rq")
        tmpq = statp.tile([128, 8], F32, tag="tmpq")
        qcols = []
        for t in range(4):
            for qi, off in enumerate((0, C)):
                if qi == 0 and t > 0:
                    continue
                qcols.append((len(qcols), t, off, 2 + 2 * qi))

        def xs_stats():
            dxA = dumpp.tile([128, C], BF16, tag="dump")
            dxB = dumpp.tile([128, C], BF16, tag="dump2")
            for t in range(4):
                ln_stats(x_sb[:, t, :], s1x[:, t : t + 1], s2x[:, t : t + 1], dxA, dxB)
            ln_mu_r(s1x, s2x, mux, rx, tmpx, 4)

        def xs_norm():
            for t in range(4):
                nc.vector.tensor_scalar(
                    out=xn[:, t, :], in0=x_sb[:, t, :],
                    scalar1=mux[:, t : t + 1], scalar2=rx[:, t : t + 1],
                    op0=OP.subtract, op1=OP.mult,
                )
                nc.vector.tensor_tensor(
                    out=xn[:, t, :], in0=xn[:, t, :], in1=reps[:, 0, :], op=OP.mult
                )
                nc.vector.tensor_tensor(
                    out=xn[:, t, :], in0=xn[:, t, :], in1=reps[:, 1, :], op=OP.add
                )

        def xs_xnT():
            for ch in range(6):
                pst = ps_t.tile([128, N], BF16, tag="pst")
                for t in range(4):
                    nc.tensor.transpose(
                        pst[:, t * 128 : (t + 1) * 128],
                        xn[:, t, ch * 128 : (ch + 1) * 128],
                        id128,
                    )
                nc.scalar.copy(out=xnT[:, ch, :], in_=pst)

        def xs_qkv(ts):
            for t in ts:
                och = [(0, 512), (512, 256)] if t == 0 else []
                och += [(768, 512), (1280, 512), (1792, 512)]
                for oi, (occ, ocs) in enumerate(och):
                    pmm = ps_mm.tile([128, N], F32, tag="mm")
                    for ch in range(6):
                        nc.tensor.matmul(
                            pmm[:, 0:ocs],
                            lhsT=xnT[:, ch, t * 128 : (t + 1) * 128],
                            rhs=wqkvt[:, ch, occ : occ + ocs],
                            start=(ch == 0), stop=False,
                        )
                    nc.tensor.matmul(
                        pmm[:, 0:ocs], lhsT=ones_col[:, 0:128],
                        rhs=bqkv[:, occ : occ + ocs], start=False, stop=True,
                    )
                    if oi % 2 == 0:
                        nc.vector.tensor_copy(
                            out=qkv[:, t, occ : occ + ocs], in_=pmm[:, 0:ocs]
                        )
                    else:
                        nc.scalar.copy(out=qkv[:, t, occ : occ + ocs], in_=pmm[:, 0:ocs])

        def xs_qkstats():
            dqA = dumpp.tile([128, C], BF16, tag="dump")
            dqB = dumpp.tile([128, C], BF16, tag="dump2")
            for col, t, off, gr in qcols:
                ln_stats(
                    qkv[:, t, off : off + C],
                    s1q[:, col : col + 1], s2q[:, col : col + 1], dqA, dqB,
                )
            ln_mu_r(s1q, s2q, muq, rq, tmpq, len(qcols))

        def xs_qknorm(sel):
            for col, t, off, gr in qcols:
                if col % 2 != sel:
                    continue
                nc.vector.tensor_scalar(
                    out=qkv[:, t, off : off + C], in0=qkv[:, t, off : off + C],
                    scalar1=muq[:, col : col + 1], scalar2=rq[:, col : col + 1],
                    op0=OP.subtract, op1=OP.mult,
                )
                nc.vector.tensor_tensor(
                    out=qkv[:, t, off : off + C], in0=qkv[:, t, off : off + C],
                    in1=reps[:, gr, :], op=OP.mult,
                )
                nc.vector.tensor_tensor(
                    out=qkv[:, t, off : off + C], in0=qkv[:, t, off : off + C],
                    in1=reps[:, gr + 1, :], op=OP.add,
                )

        def xs_kqT():
            for ch in range(6):
                pst = ps_t.tile([128, N], BF16, tag="pst")
                for t in range(4):
                    nc.tensor.transpose(
                        pst[:, t * 128 : (t + 1) * 128],
                        qkv[:, t, C + ch * 128 : C + (ch + 1) * 128],
                        id128,
                    )
                nc.scalar.copy(out=kT[:, ch, :], in_=pst)
            pst = ps_t.tile([128, N], BF16, tag="pst")
            for ch in range(6):
                nc.tensor.transpose(
                    pst[:, ch * IB : (ch + 1) * IB],
                    qkv[0:IB, 0, ch * 128 : (ch + 1) * 128],
                    id128[0:IB, 0:IB],
                )
            nc.vector.tensor_copy(
                out=qT.rearrange("p a b -> p (a b)"), in_=pst[:, 0 : 6 * IB]
            )

        # ================= fixup / attention =================
        o_sb = sing.tile([IB, H, HD], BF16)
        out_sb = sing.tile([IB, C], F32)
        mu = fixp.tile([IB, N], F32)
        var = fixp.tile([IB, N], F32)
        rr = fixp.tile([IB, N], F32)
        murb = fixp.tile([IB, N], BF16)
        rb16 = fixp.tile([IB, N], BF16)
        mwh = fixp.tile([IB, N], BF16)

        def fix_stats(lo, hi):
            nc.vector.tensor_scalar(
                out=mu[lo:hi, :], in0=braw[lo:hi, 12, :], scalar1=RC, scalar2=None,
                op0=OP.mult,
            )
            nc.vector.tensor_tensor(
                out=var[lo:hi, :], in0=mu[lo:hi, :], in1=mu[lo:hi, :], op=OP.mult
            )
            nc.vector.scalar_tensor_tensor(
                out=var[lo:hi, :], in0=braw[lo:hi, 13, :], scalar=calT[lo:hi, :],
                in1=var[lo:hi, :], op0=OP.mult, op1=OP.subtract,
            )
            nc.scalar.activation(
                out=var[lo:hi, :], in_=var[lo:hi, :], func=AF.Ln, bias=epsP[lo:hi, :]
            )
            nc.scalar.activation(
                out=rr[lo:hi, :], in_=var[lo:hi, :], func=AF.Exp, scale=-0.5,
                bias=expb[lo:hi, :],
            )
            nc.vector.tensor_tensor(
                out=murb[lo:hi, :], in0=mu[lo:hi, :], in1=rr[lo:hi, :], op=OP.mult
            )
            nc.vector.tensor_copy(out=rb16[lo:hi, :], in_=rr[lo:hi, :])

        def fix_bias(lo, hi):
            for h in range(H):
                nc.vector.tensor_scalar(
                    out=mwh[lo:hi, :], in0=murb[lo:hi, :],
                    scalar1=wsrep[lo:hi, h : h + 1], scalar2=None, op0=OP.mult,
                )
                nc.vector.tensor_tensor(
                    out=braw[lo:hi, h, :], in0=braw[lo:hi, h, :],
                    in1=mwh[lo:hi, :], op=OP.subtract,
                )
                nc.vector.tensor_tensor(
                    out=braw[lo:hi, h, :], in0=braw[lo:hi, h, :],
                    in1=rb16[lo:hi, :], op=OP.mult,
                )

        def attend_pair(lo, hi, hp):
            sp = hi - lo
            h0 = 2 * hp
            sim = attnp.tile([IB, 2, N], F32, tag="sim2")
            for e in range(2):
                h = h0 + e
                bp = (h % 2) * 64
                sps = ps_mm.tile([128, N], F32, tag="mm")
                nc.tensor.matmul(
                    sps[lo:hi, :],
                    lhsT=qT[bp : bp + 64, h // 2, lo:hi],
                    rhs=kT[bp : bp + 64, h // 2, :],
                    start=True, stop=True,
                )
                nc.vector.scalar_tensor_tensor(
                    out=sim[lo:hi, e, :],
                    in0=sps[lo:hi, :], scalar=0.125,
                    in1=braw[lo:hi, h, :],
                    op0=OP.mult, op1=OP.add,
                )
            for e in range(2):
                h = h0 + e
                esim = attnp.tile([IB, N], BF16, tag="esim")
                den = attnp.tile([IB, 1], F32, tag="den")
                nc.scalar.activation(
                    out=esim[lo:hi, :], in_=sim[lo:hi, e, :], func=AF.Exp,
                    accum_out=den[lo:hi, :],
                )
                nc.vector.reciprocal(out=den[lo:hi, :], in_=den[lo:hi, :])
                aps = ps_t.tile([128, N], BF16, tag="pst")
                for jc in range(4):
                    nc.tensor.transpose(
                        aps[:, jc * sp : (jc + 1) * sp],
                        esim[lo:hi, jc * 128 : (jc + 1) * 128],
                        id128[lo:hi, lo:hi],
                    )
                aT = attnp.tile([128, 4, IB], BF16, tag="aT")
                nc.vector.tensor_copy(
                    out=aT[:, :, 0:sp],
                    in_=aps[:, 0 : 4 * sp].rearrange("p (a b) -> p a b", a=4),
                )
                ops = ps_mm.tile([128, N], F32, tag="mm")
                for jc in range(4):
                    nc.tensor.matmul(
                        ops[lo:hi, 0:HD],
                        lhsT=aT[:, jc, 0:sp],
                        rhs=qkv[:, jc, 2 * C + h * HD : 2 * C + (h + 1) * HD],
                        start=(jc == 0), stop=(jc == 3),
                    )
                nc.vector.tensor_scalar(
                    out=o_sb[lo:hi, h, :], in0=ops[lo:hi, 0:HD],
                    scalar1=den[lo:hi, :], scalar2=None, op0=OP.mult,
                )

        def proj_half(lo, hi):
            sp = hi - lo
            o_fl = o_sb.rearrange("p a b -> p (a b)")
            oT = fixp.tile([128, 6, IB], BF16)
            pso = ps_t.tile([128, N], BF16, tag="pst")
            for ch in range(6):
                nc.tensor.transpose(
                    pso[:, ch * sp : (ch + 1) * sp],
                    o_fl[lo:hi, ch * 128 : (ch + 1) * 128],
                    id128[lo:hi, lo:hi],
                )
            nc.vector.tensor_copy(
                out=oT[:, :, 0:sp],
                in_=pso[:, 0 : 6 * sp].rearrange("p (a b) -> p a b", a=6),
            )
            for occ, ocs in [(0, 512), (512, 256)]:
                pps = ps_mm.tile([128, N], F32, tag="mm")
                for ch in range(6):
                    nc.tensor.matmul(
                        pps[lo:hi, 0:ocs],
                        lhsT=oT[:, ch, 0:sp],
                        rhs=wprojt[:, ch, occ : occ + ocs],
                        start=(ch == 0), stop=False,
                    )
                nc.tensor.matmul(
                    pps[lo:hi, 0:ocs], lhsT=ones_col[:, lo:hi],
                    rhs=bproj[:, occ : occ + ocs], start=False, stop=True,
                )
                nc.vector.tensor_copy(
                    out=out_sb[lo:hi, occ : occ + ocs], in_=pps[lo:hi, 0:ocs]
                )
            nc.sync.dma_start(out=out_d[lo:hi, :], in_=out_sb[lo:hi, :])

        # ================= emission schedule =================
        xstages = [
            xs_stats, xs_norm, xs_xnT,
            lambda: xs_qkv((0, 1)), lambda: xs_qkv((2, 3)),
            xs_qkstats, lambda: xs_qknorm(0), lambda: xs_qknorm(1), xs_kqT,
        ]
        def work_items():
            yield 17, lambda: fix_stats(0, 32)
            yield 18, lambda: fix_bias(0, 32)
            for hp in range(6):
                yield 19 + 2 * hp, (lambda hp=hp: attend_pair(0, 32, hp))
            yield 30, lambda: proj_half(0, 32)

        witems = list(work_items())
        wi = 0
        for g in range(NG):
            do_group(g)
            if g == 1:
                load_x()
            if g == 2:
                load_w1()
            if g == 8:
                load_w2()
            if 2 <= g < 2 + len(xstages):
                xstages[g - 2]()
            while wi < len(witems) and witems[wi][0] <= g:
                witems[wi][1]()
                wi += 1
        while wi < len(witems):
            witems[wi][1]()
            wi += 1
        flush_pend()
        fix_stats(32, 64)
        fix_bias(32, 64)
        for hp in range(6):
            attend_pair(32, 64, hp)
        proj_half(32, 64)

    nc.compile()
    return nc


_NC = None
_LAST_MAPS = None


def prep_maps(x, pair, ln_g, ln_b, w_qkv, b_qkv, w_proj, b_proj, w_bias,
              pn_g, pn_b, qln_g, qln_b, kln_g, kln_b):
    x = np.asarray(x, np.float32)
    pair = np.asarray(pair, np.float32)
    wqkvt = np.ascontiguousarray(np.asarray(w_qkv, np.float32).T).astype(bf)
    wprojt = np.ascontiguousarray(np.asarray(w_proj, np.float32).T).astype(bf)
    wg = (np.asarray(pn_g, np.float32)[:, None]
          * np.asarray(w_bias, np.float32).T)            # [768, 12]
    wgx = np.zeros((C, 14), np.float32)
    wgx[:, 0:H] = wg * 2.0 ** PSC
    wgx[:, 12] = 1.0
    wgx = wgx.astype(bf)
    wsum = wgx[:, 0:H].astype(np.float32).sum(axis=0)    # = wsum * 2^PSC
    wsum16 = np.zeros((1, 16), np.float32)
    wsum16[0, 0:H] = wsum
    reps = np.stack(
        [np.asarray(a, np.float32) for a in (ln_g, ln_b, qln_g, qln_b, kln_g, kln_b)]
    ).astype(bf)
    bqkv = np.asarray(b_qkv, np.float32)[None].astype(bf)
    bproj = np.asarray(b_proj, np.float32)[None].astype(bf)

    # calibration for the bit-trick square (sampled)
    samp = (np.ascontiguousarray(pair.reshape(-1)[::17]) * 2.0 ** -PSC).astype(bf)
    sbits = samp.view(np.uint16)
    ab = (((sbits & 0x7FFF).astype(np.uint32) << 1) & 0xFFFF).astype(np.uint16)
    approx = ab.view(bf).astype(np.float64)              # ~ xs^2 * 2^127
    true = samp.astype(np.float64) ** 2                  # xs^2
    cal = float(true.sum() / (approx.sum() * 2.0 ** -127))
    # device: var_s = sumsq_dev * calrc - mu_dev^2, with sumsq weight 2^-64
    calrc = np.array([[cal * RC * 2.0 ** -63]], np.float32)

    in_maps = []
    for k in range(NCORES):
        ps = pair[0, k * IB : (k + 1) * IB]  # [64, 512, 768]
        ps = np.roll(ps, -k * IB, axis=1)  # roll j to match rolled x
        pT = np.ascontiguousarray(
            ps.reshape(NG, G, 6, 128).transpose(0, 3, 2, 1) * np.float32(2.0 ** -PSC)
        ).astype(bf).reshape(NG * 128, 6 * G)
        xk = np.roll(x[0], -k * IB, axis=0).astype(bf)
        in_maps.append(
            {
                "pairT_s": pT,
                "x_s": np.ascontiguousarray(xk),
                "wqkvt": wqkvt,
                "bqkv": bqkv,
                "wprojt": wprojt,
                "bproj": bproj,
                "wgx": wgx,
                "wsum16": wsum16,
                "calrc": calrc,
                "lnreps": reps,
            }
        )

    return in_maps


def kernel(**inputs):
    global _NC, _LAST_MAPS
    if _NC is None:
        _NC = _build()
    in_maps = prep_maps(**inputs)
    _LAST_MAPS = in_maps
    res = run_bass_kernel_spmd(_NC, in_maps, list(range(NCORES)))
    outs = [res.results[k]["out"] for k in range(NCORES)]
    return np.concatenate(outs, axis=0)[None].astype(np.float32)
